# revision 1
# baseline (speedup 1.0000x reference)
"""Trainium2 Bass kernel for nn_CausalGraphNetwork (B=4,S=256,V=64,H=128,NH=8,NL=3,L=10).

Sharding: the 64 variables are split across 8 NeuronCores (8 vars each).
x / adjacency / embeddings are replicated (folded into per-core weights on host).
Each core returns its (B,S,8) prediction slice; host concatenates.
"""

import numpy as np

import concourse.bass as bass
import concourse.mybir as mybir
import concourse.tile as tile
from concourse import bacc
from concourse.bass_utils import run_bass_kernel_spmd, axon_active

F32 = mybir.dt.float32
BF16 = mybir.dt.bfloat16
AF = mybir.ActivationFunctionType
ALU = mybir.AluOpType

B, S, V, L, H, NH, NL = 4, 256, 64, 10, 128, 8, 3
DH = H // NH          # 16
LP1 = L + 1           # 11
N = B * S             # 1024 tokens
VL = V // 8           # 8 vars per core
NCORES = 8
KT = 6                # ceil(11*64/128) k-tiles for the lag-source contraction
EPS = 1e-5
HD = 2 * H            # padded head dim: 8 heads x 32 (16 real + [ones] + pad)

_CACHED = None  # (nc, input_specs) — graph is input-value independent



# ----------------------------------------------------------------------------
# Device graph
# ----------------------------------------------------------------------------
def _build_graph(stop_after=None):
    nc = bacc.Bacc(
        "TRN2", target_bir_lowering=False, debug=not axon_active(),
        num_devices=NCORES,
    )

    _LIM = {"s1": 0, "mech": 1, "proj": 2, "attn": 3, None: 99}[stop_after]

    def din(name, shape, dt=BF16):
        return nc.dram_tensor(name, list(shape), dt, kind="ExternalInput")

    xT_d = din("xT", (64, N), F32)
    U_d = din("U", (KT, 128, VL * H))
    mw_d = din("mw", (128, NL * VL * H))       # mech_W  [h, (li,i)*128+k]
    mb_d = din("mb", (128, NL * VL), F32)      # mech_b  col (li*VL+i)
    lng_d = din("lng", (128, NL * VL), F32)
    lnb_d = din("lnb", (128, NL * VL), F32)
    wq_d = din("wq", (128, VL * HD))           # padded q proj (scaled 1/4)
    wk_d = din("wk", (128, VL * HD))
    wv_d = din("wv", (128, VL * HD))           # rhs layout (K=h, N=256) per var
    bq_d = din("bq", (128, VL * 2), F32)       # per-partition bias cols (i,hg)
    bk_d = din("bk", (128, VL * 2), F32)
    bvo_d = din("bvo", (1, VL * HD))           # bv + ones-col row, per var
    ones1_d = din("ones1", (1, 128))           # all-ones K=1 lhsT
    wfp_d = din("wfp", (128, VL * 2 * 8))      # folded head lhsT per (i,hg)
    wfl_d = din("wfl", (128, VL * 2 * 8))      # l-extraction lhsT per (i,hg)
    selS_d = din("selS", (128, 64))            # stats selector: col i ones
    selB_d = din("selB", (8, VL * 2 * 128))   # bcast lhsT per (i, {RB,M2})
    selHS_d = din("selHS", (8, VL * 8))   # head-sum selector per i
    eye8_d = din("eye8", (8, 8), F32)
    bf_d = din("bfold", (8, 1), F32)

    out_d = nc.dram_tensor("preds", [N, VL], F32, kind="ExternalOutput")
    dbg_d = nc.dram_tensor("dbg", [128, N], F32, kind="ExternalOutput") \
        if stop_after else None

    with tile.TileContext(nc) as tc:
        # ---------- persistent constants (~17KB/part) ----------
        with tc.tile_pool(name="consts", bufs=1) as consts:
            mw = consts.tile([128, NL * VL * H], BF16)
            nc.sync.dma_start(out=mw, in_=mw_d[:])
            mb = consts.tile([128, NL * VL], F32)
            nc.sync.dma_start(out=mb, in_=mb_d[:])
            lng = consts.tile([128, NL * VL], F32)
            nc.sync.dma_start(out=lng, in_=lng_d[:])
            lnb = consts.tile([128, NL * VL], F32)
            nc.sync.dma_start(out=lnb, in_=lnb_d[:])
            selS = consts.tile([128, 64], BF16)
            nc.sync.dma_start(out=selS, in_=selS_d[:])
            selB = consts.tile([8, VL * 2 * 128], BF16)
            nc.sync.dma_start(out=selB, in_=selB_d[:])
            selHS = consts.tile([8, VL * 8], BF16)
            nc.sync.dma_start(out=selHS, in_=selHS_d[:])
            eye8 = consts.tile([8, 8], F32)
            nc.sync.dma_start(out=eye8, in_=eye8_d[:])
            bfold = consts.tile([8, 1], F32)
            nc.sync.dma_start(out=bfold, in_=bf_d[:])
            ones1 = consts.tile([1, 128], BF16)
            nc.sync.dma_start(out=ones1, in_=ones1_d[:])
            wfp = consts.tile([128, VL * 2 * 8], BF16)
            nc.sync.dma_start(out=wfp, in_=wfp_d[:])
            wfl = consts.tile([128, VL * 2 * 8], BF16)
            nc.sync.dma_start(out=wfl, in_=wfl_d[:])

            # oT outlives the attention pools (needed by the output head)
            with tc.tile_pool(name="atto", bufs=1) as atto:
                oT = [atto.tile([128, 512], BF16, tag=f"oT{bi}",
                                name=f"oT{bi}") for bi in range(B * VL)]

                with tc.tile_pool(name="acts", bufs=1) as acts:
                    # activations: zA[i] = (h=128, n=1024) bf16
                    zA = [acts.tile([128, N], BF16, tag=f"zA{i}",
                                    name=f"zA{i}") for i in range(VL)]

                    # ================= stage 1: causal input =================
                    with tc.tile_pool(name="s1", bufs=1) as s1, \
                         tc.tile_pool(name="s1p", bufs=2, space="PSUM") as s1p:
                        xTf = s1.tile([64, N], F32)
                        nc.sync.dma_start(out=xTf, in_=xT_d[:])
                        xbf = s1.tile([64, N], BF16)
                        nc.vector.tensor_copy(xbf, xTf)

                        Us = []
                        xlag = []
                        for k in range(KT):
                            u = s1.tile([128, VL * H], BF16, tag=f"U{k}",
                                        name=f"U{k}")
                            nc.sync.dma_start(out=u, in_=U_d[k])
                            Us.append(u)
                            xl = s1.tile([128, N], BF16, tag=f"xl{k}",
                                         name=f"xl{k}")
                            nc.vector.memset(xl, 0.0)
                            xlag.append(xl)

                        # xlag[k][dl*64+s, b*256+t] = x[b, t-(2k+dl), s]
                        for k in range(KT):
                            for dl in range(2):
                                lag = 2 * k + dl
                                if lag > L:
                                    continue
                                if lag == 0:
                                    nc.sync.dma_start(out=xlag[k][0:64, :],
                                                      in_=xbf[:, :])
                                else:
                                    src = xbf.rearrange(
                                        "s (b t) -> s b t", b=B)[:, :, 0:S - lag]
                                    dst = xlag[k][dl * 64:dl * 64 + 64, :]\
                                        .rearrange("s (b t) -> s b t", b=B)[
                                            :, :, lag:S]
                                    nc.sync.dma_start(out=dst, in_=src)

                        for i in range(VL):
                            zp = s1p.tile([128, N], F32, tag="zp")
                            for c in range(2):
                                for k in range(KT):
                                    nc.tensor.matmul(
                                        zp[:, c * 512:(c + 1) * 512],
                                        lhsT=Us[k][:, i * H:(i + 1) * H],
                                        rhs=xlag[k][:, c * 512:(c + 1) * 512],
                                        start=(k == 0), stop=(k == KT - 1),
                                    )
                            nc.vector.tensor_copy(zA[i], zp)

                    if stop_after == "s1":
                        with tc.tile_pool(name="dbgp", bufs=1) as dbgp:
                            dv = dbgp.tile([128, N], F32)
                            nc.vector.tensor_copy(dv, zA[0])
                            nc.sync.dma_start(out=dbg_d[:], in_=dv)

                    # ================= mech layers =================
                    with tc.tile_pool(name="mp", bufs=1) as mp, \
                         tc.tile_pool(name="mt", bufs=2) as mt, \
                         tc.tile_pool(name="mpp", bufs=1,
                                      space="PSUM") as mpp, \
                         tc.tile_pool(name="msp", bufs=1,
                                      space="PSUM") as msp:
                        for li in range(NL if _LIM >= 1 else 0):
                            statsP = msp.tile([8, 2 * N], F32, tag="statsP")
                            zsbs = []
                            for i in range(VL):
                                ci = li * VL + i
                                zp = mpp.tile([128, N], F32, tag="zp",
                                              bufs=2)
                                for c in range(2):
                                    nc.tensor.matmul(
                                        zp[:, c * 512:(c + 1) * 512],
                                        lhsT=mw[:, ci * H:(ci + 1) * H],
                                        rhs=zA[i][:, c * 512:(c + 1) * 512],
                                        start=True, stop=True,
                                    )
                                zsb = mp.tile([128, N], BF16, tag=f"zsb{i}",
                                              name=f"zsb{i}")
                                nc.scalar.activation(
                                    zsb, zp, AF.Identity,
                                    bias=mb[:, ci:ci + 1], scale=1.0)
                                zsq = mt.tile([128, N], BF16, tag="zsq")
                                nc.vector.tensor_mul(zsq, zsb, zsb)
                                zsbs.append(zsb)
                                for c in range(2):
                                    nc.tensor.matmul(
                                        statsP[:, c * 512:(c + 1) * 512],
                                        lhsT=selS[:, i * 8:(i + 1) * 8],
                                        rhs=zsb[:, c * 512:(c + 1) * 512],
                                        start=(i == 0), stop=(i == VL - 1),
                                    )
                                    nc.tensor.matmul(
                                        statsP[:, N + c * 512:N + (c + 1) * 512],
                                        lhsT=selS[:, i * 8:(i + 1) * 8],
                                        rhs=zsq[:, c * 512:(c + 1) * 512],
                                        start=(i == 0), stop=(i == VL - 1),
                                    )

                            # r8 = 1/sqrt(128*sumsq - sum^2 + 16384*eps)
                            stS = mp.tile([8, 2 * N], F32)
                            nc.vector.tensor_copy(stS, statsP)
                            t1 = mp.tile([8, N], F32)
                            nc.vector.tensor_mul(t1, stS[:, 0:N], stS[:, 0:N])
                            t2 = mp.tile([8, N], F32)
                            nc.vector.tensor_scalar(
                                out=t2, in0=stS[:, N:2 * N], scalar1=float(H),
                                scalar2=None, op0=ALU.mult)
                            nc.vector.tensor_tensor(
                                out=t2, in0=t2, in1=t1, op=ALU.subtract)
                            st2 = mp.tile([8, 2 * N], BF16)
                            rfp = mp.tile([8, N], F32)
                            sq = mp.tile([8, N], F32)
                            epsb = mp.tile([8, 1], F32)
                            nc.vector.memset(epsb, float(H * H * EPS))
                            nc.scalar.activation(sq, t2, AF.Sqrt,
                                                 bias=epsb, scale=1.0)
                            nc.vector.reciprocal_approx_fast(
                                out=rfp, in_=sq)
                            nc.vector.tensor_copy(st2[:, 0:N], rfp)
                            nc.vector.tensor_tensor(
                                out=st2[:, N:2 * N], in0=stS[:, 0:N],
                                in1=rfp, op=ALU.mult)

                            for i in range(VL):
                                ci = li * VL + i
                                bcp = msp.tile([128, N], F32, tag="statsP")
                                for c in range(2):
                                    nc.tensor.matmul(
                                        bcp[:, c * 512:(c + 1) * 512],
                                        lhsT=selB[:, (i * 2) * 128:
                                                  (i * 2 + 1) * 128],
                                        rhs=st2[:, c * 512:(c + 1) * 512],
                                        start=True, stop=True,
                                    )
                                bcs = mt.tile([128, N], BF16, tag="bcs")
                                nc.vector.tensor_copy(bcs, bcp)
                                t4 = mt.tile([128, N], BF16, tag="t4")
                                nc.gpsimd.tensor_tensor(
                                    out=t4, in0=zsbs[i], in1=bcs,
                                    op=ALU.mult)
                                bcp2 = msp.tile([128, N], F32, tag="statsP")
                                for c in range(2):
                                    nc.tensor.matmul(
                                        bcp2[:, c * 512:(c + 1) * 512],
                                        lhsT=selB[:, (i * 2 + 1) * 128:
                                                  (i * 2 + 2) * 128],
                                        rhs=st2[:, N + c * 512:
                                                N + (c + 1) * 512],
                                        start=True, stop=True,
                                    )
                                bcs2 = mt.tile([128, N], BF16, tag="bcs2")
                                nc.vector.tensor_copy(bcs2, bcp2)
                                nc.vector.tensor_tensor(
                                    out=t4, in0=t4, in1=bcs2,
                                    op=ALU.add)
                                nc.scalar.activation(
                                    zA[i], t4, AF.Gelu,
                                    bias=lnb[:, ci:ci + 1],
                                    scale=lng[:, ci:ci + 1])

                    if stop_after == "mech":
                        with tc.tile_pool(name="dbgp", bufs=1) as dbgp:
                            dv = dbgp.tile([128, N], F32)
                            nc.vector.tensor_copy(dv, zA[0])
                            nc.sync.dma_start(out=dbg_d[:], in_=dv)

                    # ============== attention: projections + core ==============
                    with tc.tile_pool(name="attd", bufs=1) as attd:
                      if _LIM >= 2:
                        wq = attd.tile([128, VL * HD], BF16)
                        nc.sync.dma_start(out=wq, in_=wq_d[:])
                        wk = attd.tile([128, VL * HD], BF16)
                        nc.sync.dma_start(out=wk, in_=wk_d[:])
                        wv = attd.tile([128, VL * HD], BF16)
                        nc.sync.dma_start(out=wv, in_=wv_d[:])
                        bq = attd.tile([128, VL * 2], F32)
                        nc.sync.dma_start(out=bq, in_=bq_d[:])
                        bk = attd.tile([128, VL * 2], F32)
                        nc.sync.dma_start(out=bk, in_=bk_d[:])
                        bvo = attd.tile([1, VL * HD], BF16)
                        nc.sync.dma_start(out=bvo, in_=bvo_d[:])

                        qT = [attd.tile([128, 2 * N], BF16, tag=f"qT{i}",
                                        name=f"qT{i}") for i in range(VL)]
                        kTt = [attd.tile([128, 2 * N], BF16, tag=f"kT{i}",
                                         name=f"kT{i}") for i in range(VL)]
                        vB = [attd.tile([128, 2 * N], BF16, tag=f"vB{i}",
                                        name=f"vB{i}") for i in range(VL)]
                        with tc.tile_pool(name="prj", bufs=1,
                                          space="PSUM") as prj:
                            for i in range(VL):
                                for (wmat, bias, dstl) in (
                                        (wq, bq, qT), (wk, bk, kTt)):
                                    pp = prj.tile([128, 2 * N], F32, tag="pp")
                                    for hg in range(2):
                                        for c in range(2):
                                            nc.tensor.matmul(
                                                pp[:, hg * N + c * 512:
                                                   hg * N + (c + 1) * 512],
                                                lhsT=wmat[
                                                    :, i * HD + hg * 128:
                                                    i * HD + (hg + 1) * 128],
                                                rhs=zA[i][:, c * 512:
                                                          (c + 1) * 512],
                                                start=True, stop=True,
                                            )
                                    for hg in range(2):
                                        nc.scalar.activation(
                                            dstl[i][:, hg * N:(hg + 1) * N],
                                            pp[:, hg * N:(hg + 1) * N],
                                            AF.Identity,
                                            bias=bias[:, i * 2 + hg:
                                                      i * 2 + hg + 1],
                                            scale=1.0)
                                # v in (token, padded-head) layout + bv + ones
                                vp = prj.tile([128, 2 * N], F32, tag="vp")
                                for nt in range(8):
                                    nc.tensor.matmul(
                                        vp[:, nt * 256:(nt + 1) * 256],
                                        lhsT=zA[i][:, nt * 128:(nt + 1) * 128],
                                        rhs=wv[:, i * HD:(i + 1) * HD],
                                        start=True, stop=False,
                                    )
                                    nc.tensor.matmul(
                                        vp[:, nt * 256:(nt + 1) * 256],
                                        lhsT=ones1,
                                        rhs=bvo[:, i * HD:(i + 1) * HD],
                                        start=False, stop=True,
                                    )
                                nc.vector.tensor_copy(vB[i], vp)

                        if stop_after == "proj":
                            with tc.tile_pool(name="dbgp", bufs=1) as dbgp:
                                dv = dbgp.tile([128, N], F32)
                                nc.vector.tensor_copy(dv, qT[0][:, 0:N])
                                nc.sync.dma_start(out=dbg_d[:], in_=dv)

                        # ---- attention core ----
                        if _LIM >= 3:
                         with tc.tile_pool(name="att", bufs=4) as att, \
                             tc.tile_pool(name="scp", bufs=2,
                                          space="PSUM") as scp:
                            for b in range(B):
                                for i in range(VL):
                                    exps = []
                                    for kt in range(2):
                                        scP = scp.tile([128, 2048], F32,
                                                       tag="scP")
                                        for hh in range(4):
                                            for hg in range(2):
                                                jb = 2 * hh + hg
                                                nc.tensor.matmul(
                                                    scP[:, jb * 256:
                                                        (jb + 1) * 256],
                                                    lhsT=kTt[i][
                                                        32 * hh:32 * hh + 32,
                                                        hg * N + b * 256
                                                        + kt * 128:
                                                        hg * N + b * 256
                                                        + (kt + 1) * 128],
                                                    rhs=qT[i][
                                                        32 * hh:32 * hh + 32,
                                                        hg * N + b * 256:
                                                        hg * N + (b + 1) * 256],
                                                    start=True, stop=True,
                                                    tile_position=(32 * hh, 0),
                                                )
                                        ex = att.tile([128, 2048], BF16,
                                                      tag="ex")
                                        nc.scalar.activation(ex, scP, AF.Exp)
                                        exps.append(ex)
                                    otP = scp.tile([128, 512], F32,
                                                   tag="scP")
                                    for hg in range(2):
                                        for hh in range(4):
                                            j = 4 * hg + hh
                                            jb = 2 * hh + hg
                                            for kt in range(2):
                                                v0 = (b * 2 + kt) * 256 + j * 32
                                                nc.tensor.matmul(
                                                    otP[32 * hh:32 * hh + 32,
                                                        hg * 256:
                                                        (hg + 1) * 256],
                                                    lhsT=vB[i][:, v0:v0 + 32],
                                                    rhs=exps[kt][
                                                        :, jb * 256:
                                                        (jb + 1) * 256],
                                                    start=(kt == 0),
                                                    stop=(kt == 1),
                                                    tile_position=(0, 32 * hh),
                                                )
                                    nc.vector.tensor_copy(oT[b * VL + i], otP)

                if stop_after == "attn":
                    with tc.tile_pool(name="dbgp", bufs=1) as dbgp:
                        dv = dbgp.tile([128, N], F32)
                        for bi in range(2):
                            nc.vector.tensor_copy(
                                dv[:, bi * 512:(bi + 1) * 512], oT[bi])
                        nc.sync.dma_start(out=dbg_d[:], in_=dv)

                # ================= output head =================
                if _LIM >= 4:
                 with tc.tile_pool(name="ph", bufs=2) as php, \
                     tc.tile_pool(name="phpp", bufs=1, space="PSUM") as phpp:
                    outst = php.tile([128, 64], F32)
                    for b in range(B):
                        PHp = phpp.tile([8, 2048], F32, tag="PHp")
                        PHl = phpp.tile([8, 2048], F32, tag="PHl")
                        for i in range(VL):
                            for hg in range(2):
                                nc.tensor.matmul(
                                    PHp[:, i * 256:(i + 1) * 256],
                                    lhsT=wfp[:, (i * 2 + hg) * 8:
                                             (i * 2 + hg + 1) * 8],
                                    rhs=oT[b * VL + i][:, hg * 256:
                                                       (hg + 1) * 256],
                                    start=(hg == 0), stop=(hg == 1),
                                )
                                nc.tensor.matmul(
                                    PHl[:, i * 256:(i + 1) * 256],
                                    lhsT=wfl[:, (i * 2 + hg) * 8:
                                             (i * 2 + hg + 1) * 8],
                                    rhs=oT[b * VL + i][:, hg * 256:
                                                       (hg + 1) * 256],
                                    start=(hg == 0), stop=(hg == 1),
                                )
                        PHs = php.tile([8, 2048], F32, tag="PHs")
                        nc.scalar.activation(PHs, PHp, AF.Copy)
                        rl = php.tile([8, 2048], F32, tag="rl")
                        nc.vector.reciprocal_approx_fast(out=rl, in_=PHl)
                        pn = php.tile([8, 2048], BF16, tag="pn")
                        nc.vector.tensor_tensor(
                            out=pn, in0=PHs, in1=rl, op=ALU.mult)
                        pP = phpp.tile([8, 256], F32, tag="PHp")
                        for i in range(VL):
                            nc.tensor.matmul(
                                pP[:, :],
                                lhsT=selHS[:, i * 8:(i + 1) * 8],
                                rhs=pn[:, i * 256:(i + 1) * 256],
                                start=(i == 0), stop=(i == VL - 1),
                            )
                        pSb = php.tile([8, 256], F32, tag="pSb")
                        nc.vector.tensor_scalar(
                            out=pSb, in0=pP, scalar1=bfold[:, 0:1],
                            scalar2=None, op0=ALU.add)
                        for c in range(2):
                            tp = phpp.tile([128, 8], F32, tag="PHl")
                            nc.tensor.transpose(
                                tp, pSb[:, c * 128:(c + 1) * 128], eye8)
                            nc.vector.tensor_copy(
                                outst[:, (b * 2 + c) * 8:(b * 2 + c + 1) * 8],
                                tp)
                    nc.sync.dma_start(
                        out=out_d.rearrange("(cb p) v -> p cb v", p=128),
                        in_=outst.rearrange("p (cb v) -> p cb v", v=VL))

    nc.compile()
    return nc


# ----------------------------------------------------------------------------
# Host-side preprocessing
# ----------------------------------------------------------------------------
def _prep_core(core, adj, var_emb, temp_emb, mech_W, mech_b, ln_g, ln_b,
               Wq, Wk, Wv, Wo, bq, bk, bv, bo, out_W, out_b, xT):
    b16 = np.float32  # cast later via ml_dtypes-free path (bass converts? no!)
    import ml_dtypes
    bf = ml_dtypes.bfloat16
    gi = slice(core * VL, (core + 1) * VL)

    # U[(k), dl*64+s, i*H+h] = adj[s, g, 2k+dl] * (var_emb[s,h]+temp_emb[l,h])
    U = np.zeros((KT, 128, VL * H), np.float32)
    for k in range(KT):
        for dl in range(2):
            lag = 2 * k + dl
            if lag > L:
                continue
            emb = var_emb + temp_emb[lag][None, :]          # (64, H)
            a = adj[:, gi, lag]                             # (64, VL)
            U[k, dl * 64:dl * 64 + 64, :] = (
                a[:, :, None] * emb[:, None, :]).reshape(64, VL * H)

    mw = np.zeros((128, NL * VL * H), np.float32)
    mb = np.zeros((128, NL * VL), np.float32)
    lng = np.zeros((128, NL * VL), np.float32)
    lnb = np.zeros((128, NL * VL), np.float32)
    for li in range(NL):
        for i in range(VL):
            g = core * VL + i
            ci = li * VL + i
            mw[:, ci * H:(ci + 1) * H] = mech_W[g, li]
            mb[:, ci] = mech_b[g, li]
            lng[:, ci] = ln_g[g, li]
            lnb[:, ci] = ln_b[g, li]

    def padw(Wm, scale=1.0):
        # (VL,H,H) -> (128, VL*HD): col i*HD + 32*j + d = W[:, 16j+d]*scale
        out = np.zeros((128, VL * HD), np.float32)
        for i in range(VL):
            g = core * VL + i
            for j in range(NH):
                out[:, i * HD + 32 * j:i * HD + 32 * j + DH] = \
                    Wm[g][:, DH * j:DH * (j + 1)] * scale
        return out

    wqp = padw(Wq, 1.0 / np.sqrt(DH))
    wkp = padw(Wk)
    wvp = padw(Wv)

    def padb(bm, scale=1.0):
        # (VL,H) -> (128, VL*2) per-partition cols by (i, hg)
        out = np.zeros((128, VL * 2), np.float32)
        for i in range(VL):
            g = core * VL + i
            for j in range(NH):
                hg, hh = divmod(j, 4)
                out[32 * hh:32 * hh + DH, i * 2 + hg] = \
                    bm[g][DH * j:DH * (j + 1)] * scale
        return out

    bqp = padb(bq, 1.0 / np.sqrt(DH))
    bkp = padb(bk)
    bvo = np.zeros((1, VL * HD), np.float32)
    for i in range(VL):
        g = core * VL + i
        for j in range(NH):
            bvo[0, i * HD + 32 * j:i * HD + 32 * j + DH] = \
                bv[g][DH * j:DH * (j + 1)]
            bvo[0, i * HD + 32 * j + DH] = 1.0   # ones column -> softmax denom

    wfold = np.einsum('vhk,vk->vh', Wo[gi], out_W[gi])      # (VL, H)
    wfp = np.zeros((128, VL * 2 * 8), np.float32)
    wfl = np.zeros((128, VL * 2 * 8), np.float32)
    for i in range(VL):
        for hg in range(2):
            m0 = (i * 2 + hg) * 8
            for hh in range(4):
                j = 4 * hg + hh
                wfp[32 * hh:32 * hh + DH, m0 + j] = \
                    wfold[i, DH * j:DH * (j + 1)]
                wfl[32 * hh + DH, m0 + j] = 1.0             # l extraction
    bfold = (np.einsum('vh,vh->v', bo[gi], out_W[gi]) +
             out_b[gi]).astype(np.float32).reshape(8, 1)

    selS = np.zeros((128, 64), np.float32)
    for j in range(8):
        selS[:, j * 8 + j] = 1.0
    selB = np.zeros((8, VL * 2 * 128), np.float32)
    for i in range(VL):
        selB[i, (i * 2) * 128:(i * 2 + 1) * 128] = float(H)
        selB[i, (i * 2 + 1) * 128:(i * 2 + 2) * 128] = -1.0
    selHS = np.zeros((8, VL * 8), np.float32)
    for i in range(VL):
        selHS[:, i * 8 + i] = 1.0
    ones1 = np.ones((1, 128), np.float32)
    eye8 = np.eye(8, dtype=np.float32)

    bfc = lambda a: a.astype(bf)
    return {
        "xT": xT, "U": bfc(U), "mw": bfc(mw), "mb": mb, "lng": lng, "lnb": lnb,
        "wq": bfc(wqp), "wk": bfc(wkp), "wv": bfc(wvp), "bq": bqp, "bk": bkp,
        "bvo": bfc(bvo), "ones1": bfc(ones1), "wfp": bfc(wfp),
        "wfl": bfc(wfl), "selS": bfc(selS),
        "selB": bfc(selB), "selHS": bfc(selHS), "eye8": eye8, "bfold": bfold,
    }


def _run(inputs, trace=False):
    global _CACHED
    if _CACHED is None:
        _CACHED = _build_graph()
    nc = _CACHED

    f = lambda t: np.asarray(t, np.float32)
    x = f(inputs["x"])
    adj = 1.0 / (1.0 + np.exp(-f(inputs["adjacency_logits"])))
    xT = np.ascontiguousarray(
        x.reshape(N, V).T).astype(np.float32)            # (64, 1024)

    args = dict(
        adj=adj, var_emb=f(inputs["var_emb"]), temp_emb=f(inputs["temp_emb"]),
        mech_W=f(inputs["mech_W"]), mech_b=f(inputs["mech_b"]),
        ln_g=f(inputs["ln_g"]), ln_b=f(inputs["ln_b"]),
        Wq=f(inputs["Wq"]), Wk=f(inputs["Wk"]), Wv=f(inputs["Wv"]),
        Wo=f(inputs["Wo"]), bq=f(inputs["bq"]), bk=f(inputs["bk"]),
        bv=f(inputs["bv"]), bo=f(inputs["bo"]),
        out_W=f(inputs["out_W"]), out_b=f(inputs["out_b"]), xT=xT,
    )
    in_maps = [_prep_core(c, **args) for c in range(NCORES)]
    res = run_bass_kernel_spmd(nc, in_maps, list(range(NCORES)), trace=trace)
    preds = np.concatenate(
        [res.results[c]["preds"].reshape(B, S, VL) for c in range(NCORES)],
        axis=2).astype(np.float32)
    return preds, res


def kernel(**inputs):
    preds, _ = _run(inputs, trace=False)
    return preds



# revision 20
# speedup vs baseline: 1.1058x; 1.1058x over previous
"""Trainium2 Bass kernel for nn_CausalGraphNetwork (B=4,S=256,V=64,H=128,NH=8,NL=3,L=10).

Sharding: the 64 variables are split across 8 NeuronCores (8 vars each).
x / adjacency / embeddings are replicated (folded into per-core weights on host).
Each core returns its (B,S,8) prediction slice; host concatenates.

Mech layers use an SBUF-path LayerNorm: per-token mean/rstd rows are computed
with column-packed PE matmul chains, broadcast across partitions via DMA, and
applied with two bf16 vector ops; gelu carries ln_g/ln_b in its scale/bias
slots. Stage-1's causal-input contraction is folded with mech layer 0's weight
(U' = U @ W0) so layer 0's pre-LN activation comes straight out of stage 1.
"""

import numpy as np

import concourse.bass as bass
import concourse.mybir as mybir
import concourse.tile as tile
from concourse import bacc
from concourse.bass_utils import run_bass_kernel_spmd, axon_active

F32 = mybir.dt.float32
BF16 = mybir.dt.bfloat16
AF = mybir.ActivationFunctionType
ALU = mybir.AluOpType

B, S, V, L, H, NH, NL = 4, 256, 64, 10, 128, 8, 3
DH = H // NH          # 16
LP1 = L + 1           # 11
N = B * S             # 1024 tokens
VL = V // 8           # 8 vars per core
NCORES = 8
KT = 6                # ceil(11*64/128) k-tiles for the lag-source contraction
EPS = 1e-5
HD = 2 * H            # padded head dim: 8 heads x 32 (16 real + [ones] + pad)

_CACHED = None  # (nc, input_specs) — graph is input-value independent
_DEBUG = False


# ----------------------------------------------------------------------------
# Device graph
# ----------------------------------------------------------------------------
def _build_graph():
    nc = bacc.Bacc(
        "TRN2", target_bir_lowering=False, debug=not axon_active(),
        num_devices=NCORES,
    )

    def din(name, shape, dt=BF16):
        return nc.dram_tensor(name, list(shape), dt, kind="ExternalInput")

    xT_d = din("xT", (64, N), F32)
    Up_d = din("Up", (KT, 128, VL * H))        # (U @ W0) + bias ones-row
    mw2_d = din("mw2", (128, 2 * VL * H))      # mech_W l1,l2
    mb2_d = din("mb2", (128, 2 * VL), F32)     # mech_b l1,l2 (cast bias)
    lng_d = din("lng", (128, NL * VL), F32)
    lnb_d = din("lnb", (128, NL * VL), F32)
    selMu_d = din("selMu", (128, 64))          # per-var mean selector (1/H)
    selSS_d = din("selSS", (128, 64))          # per-var sumsq selector (1/H)
    wq_d = din("wq", (128, VL * HD))           # padded q proj (scaled 1/4)
    wk_d = din("wk", (128, VL * HD))
    wv_d = din("wv", (128, VL * HD))           # rhs layout (K=h, N=256) per var
    bq_d = din("bq", (128, VL * 2), F32)       # per-partition bias cols (i,hg)
    bk_d = din("bk", (128, VL * 2), F32)
    bvo_d = din("bvo", (1, VL * HD))           # bv + ones-col row, per var
    ones1_d = din("ones1", (1, 128))           # all-ones K=1 lhsT
    wfp_d = din("wfp", (128, VL * 2 * 8))      # folded head lhsT per (i,hg)
    wfl_d = din("wfl", (128, VL * 2 * 8))      # l-extraction lhsT per (i,hg)
    selHS_d = din("selHS", (8, VL * 8))        # head-sum selector per i
    eye8_d = din("eye8", (8, 8), F32)
    bf_d = din("bfold", (8, 1), F32)

    out_d = nc.dram_tensor("preds", [N, VL], F32, kind="ExternalOutput")
    # DRAM staging for the per-token LN rows (partition-broadcast via DMA)
    rmu_d = nc.dram_tensor("rmu", [NL, 2, 8, N], BF16, kind="Internal")
    dbg_d = nc.dram_tensor("dbg", [4, 128, N], F32, kind="ExternalOutput") \
        if _DEBUG else None

    with tile.TileContext(nc) as tc:
        # ---------- persistent constants ----------
        with tc.tile_pool(name="consts", bufs=1) as consts:
            mw2 = consts.tile([128, 2 * VL * H], BF16)
            nc.sync.dma_start(out=mw2, in_=mw2_d[:])
            mb2 = consts.tile([128, 2 * VL], F32)
            nc.sync.dma_start(out=mb2, in_=mb2_d[:])
            lng = consts.tile([128, NL * VL], F32)
            nc.sync.dma_start(out=lng, in_=lng_d[:])
            lnb = consts.tile([128, NL * VL], F32)
            nc.sync.dma_start(out=lnb, in_=lnb_d[:])
            selMu = consts.tile([128, 64], BF16)
            nc.sync.dma_start(out=selMu, in_=selMu_d[:])
            selSS = consts.tile([128, 64], BF16)
            nc.sync.dma_start(out=selSS, in_=selSS_d[:])
            selHS = consts.tile([8, VL * 8], BF16)
            nc.sync.dma_start(out=selHS, in_=selHS_d[:])
            eye8 = consts.tile([8, 8], F32)
            nc.sync.dma_start(out=eye8, in_=eye8_d[:])
            bfold = consts.tile([8, 1], F32)
            nc.sync.dma_start(out=bfold, in_=bf_d[:])
            ones1 = consts.tile([1, 128], BF16)
            nc.sync.dma_start(out=ones1, in_=ones1_d[:])
            wfp = consts.tile([128, VL * 2 * 8], BF16)
            nc.sync.dma_start(out=wfp, in_=wfp_d[:])
            wfl = consts.tile([128, VL * 2 * 8], BF16)
            nc.sync.dma_start(out=wfl, in_=wfl_d[:])

            # oT outlives the attention pools (needed by the output head)
            with tc.tile_pool(name="atto", bufs=1) as atto:
                oT = [atto.tile([128, 512], BF16, tag=f"oT{bi}",
                                name=f"oT{bi}") for bi in range(B * VL)]

                with tc.tile_pool(name="acts", bufs=1) as acts:
                    # final activations consumed by attention projections
                    zA = [acts.tile([128, N], BF16, tag=f"zA{i}",
                                    name=f"zA{i}") for i in range(VL)]

                    # ============ stage 1 + mech layers (fused LN) ============
                    with tc.tile_pool(name="ms", bufs=1) as ms, \
                         tc.tile_pool(name="msq", bufs=3) as msq, \
                         tc.tile_pool(name="mbc", bufs=3) as mbc, \
                         tc.tile_pool(name="mtt", bufs=2) as mtt, \
                         tc.tile_pool(name="mzp", bufs=3, space="PSUM") as mzp, \
                         tc.tile_pool(name="mst", bufs=1, space="PSUM") as mst:
                        # stage-1 lagged input
                        xTf = ms.tile([64, N], F32)
                        nc.sync.dma_start(out=xTf, in_=xT_d[:])
                        xbf = ms.tile([64, N], BF16)
                        nc.vector.tensor_copy(xbf, xTf)

                        Us = []
                        xlag = []
                        for k in range(KT):
                            u = ms.tile([128, VL * H], BF16, name=f"U{k}")
                            nc.sync.dma_start(out=u, in_=Up_d[k])
                            Us.append(u)
                            xl = ms.tile([128, N], BF16, name=f"xl{k}")
                            nc.vector.memset(xl, 0.0)
                            xlag.append(xl)
                        # ones row for the folded l0 bias
                        nc.vector.memset(xlag[KT - 1][64:65, :], 1.0)

                        # xlag[k][dl*64+s, b*256+t] = x[b, t-(2k+dl), s]
                        for k in range(KT):
                            for dl in range(2):
                                lag = 2 * k + dl
                                if lag > L:
                                    continue
                                if lag == 0:
                                    nc.sync.dma_start(out=xlag[k][0:64, :],
                                                      in_=xbf[:, :])
                                else:
                                    src = xbf.rearrange(
                                        "s (b t) -> s b t", b=B)[:, :, 0:S - lag]
                                    dst = xlag[k][dl * 64:dl * 64 + 64, :]\
                                        .rearrange("s (b t) -> s b t", b=B)[
                                            :, :, lag:S]
                                    nc.sync.dma_start(out=dst, in_=src)

                        pa = [ms.tile([128, N], BF16, name=f"pA{i}")
                              for i in range(VL)]
                        pb = [ms.tile([128, N], BF16, name=f"pB{i}")
                              for i in range(VL)]
                        zsbA = [ms.tile([128, N], BF16, name=f"zsbA{i}")
                                for i in range(VL)]
                        zsbB = [ms.tile([128, N], BF16, name=f"zsbB{i}")
                                for i in range(VL)]
                        # stats rows (per layer, reused across layers)
                        mt = ms.tile([8, N], F32, name="mt")
                        v1 = ms.tile([8, N], F32, name="v1")
                        varr = ms.tile([8, N], F32, name="varr")
                        sd = ms.tile([8, N], F32, name="sd")
                        rrf = ms.tile([8, N], F32, name="rrf")
                        rbb = ms.tile([8, N], BF16, name="rbb")
                        mnb = ms.tile([8, N], BF16, name="mnb")
                        epsb = ms.tile([8, 1], F32, name="epsb")
                        nc.vector.memset(epsb, EPS)

                        for li in range(NL):
                            zin = [None, pa, pb][li]
                            zout = [pa, pb, zA][li]
                            zsb = zsbA if li % 2 == 0 else zsbB
                            statsT = mst.tile([40, N], F32, tag="st")
                            for i in range(VL):
                                zp = mzp.tile([128, N], F32, tag="zp")
                                if li == 0:
                                    for c in range(2):
                                        cs = slice(c * 512, (c + 1) * 512)
                                        for k in range(KT):
                                            nc.tensor.matmul(
                                                zp[:, cs],
                                                lhsT=Us[k][:, i * H:(i + 1) * H],
                                                rhs=xlag[k][:, cs],
                                                start=(k == 0),
                                                stop=(k == KT - 1),
                                            )
                                    nc.vector.tensor_copy(zsb[i], zp)
                                else:
                                    ci = (li - 1) * VL + i
                                    for c in range(2):
                                        cs = slice(c * 512, (c + 1) * 512)
                                        nc.tensor.matmul(
                                            zp[:, cs],
                                            lhsT=mw2[:, ci * H:(ci + 1) * H],
                                            rhs=zin[i][:, cs],
                                            start=True, stop=True,
                                        )
                                    nc.scalar.activation(
                                        zsb[i], zp, AF.Identity,
                                        bias=mb2[:, ci:ci + 1], scale=1.0)
                                zsq = msq.tile([128, N], BF16, tag="zsq")
                                if i % 2 == 0:
                                    nc.scalar.activation(zsq, zsb[i],
                                                         AF.Square)
                                else:
                                    nc.gpsimd.tensor_mul(zsq, zsb[i], zsb[i])
                                for c in range(2):
                                    cs = slice(c * 512, (c + 1) * 512)
                                    nc.tensor.matmul(
                                        statsT[0:8, cs],
                                        lhsT=selMu[:, i * 8:(i + 1) * 8],
                                        rhs=zsb[i][:, cs],
                                        start=(i == 0), stop=(i == VL - 1),
                                        tile_position=(0, 0),
                                    )
                                    nc.tensor.matmul(
                                        statsT[32:40, cs],
                                        lhsT=selSS[:, i * 8:(i + 1) * 8],
                                        rhs=zsq[:, cs],
                                        start=(i == 0), stop=(i == VL - 1),
                                        tile_position=(0, 32),
                                    )

                            # per-token stats -> rstd / -mean rows
                            nc.vector.tensor_copy(mt, statsT[0:8, :])
                            nc.gpsimd.tensor_mul(v1, mt, mt)
                            nc.vector.scalar_tensor_tensor(
                                out=varr, in0=statsT[32:40, :], scalar=1.0,
                                in1=v1, op0=ALU.mult, op1=ALU.subtract)
                            nc.scalar.activation(sd, varr, AF.Sqrt,
                                                 bias=epsb[:, 0:1], scale=1.0)
                            nc.vector.reciprocal_approx_fast(out=rrf, in_=sd)
                            nc.vector.tensor_copy(rbb, rrf)
                            nc.vector.tensor_scalar(
                                out=mnb, in0=mt, scalar1=-1.0, scalar2=None,
                                op0=ALU.mult)
                            nc.sync.dma_start(out=rmu_d[li, 0], in_=rbb)
                            nc.sync.dma_start(out=rmu_d[li, 1], in_=mnb)

                            # normalize + gelu
                            for i in range(VL):
                                ci = li * VL + i
                                RB = mbc.tile([128, N], BF16, tag="RB")
                                nc.sync.dma_start(
                                    out=RB,
                                    in_=rmu_d[li, 0, i:i + 1, :]
                                    .partition_broadcast(128))
                                MUB = mbc.tile([128, N], BF16, tag="MUB")
                                nc.sync.dma_start(
                                    out=MUB,
                                    in_=rmu_d[li, 1, i:i + 1, :]
                                    .partition_broadcast(128))
                                ta = mtt.tile([128, N], BF16, tag="ta")
                                nc.vector.tensor_add(ta, zsb[i], MUB)
                                t5 = mtt.tile([128, N], BF16, tag="t5")
                                nc.vector.tensor_mul(t5, ta, RB)
                                nc.scalar.activation(
                                    zout[i], t5, AF.Gelu,
                                    bias=lnb[:, ci:ci + 1],
                                    scale=lng[:, ci:ci + 1])
                                if _DEBUG and li >= 1 and i == 0:
                                    dv = ms.tile([128, N], F32,
                                                 name=f"dbg{li}")
                                    nc.vector.tensor_copy(dv, zout[0])
                                    nc.sync.dma_start(out=dbg_d[li - 1],
                                                      in_=dv)

                    # ============== attention: projections + core ==============
                    with tc.tile_pool(name="attd", bufs=1) as attd:
                        wq = attd.tile([128, VL * HD], BF16)
                        nc.sync.dma_start(out=wq, in_=wq_d[:])
                        wk = attd.tile([128, VL * HD], BF16)
                        nc.sync.dma_start(out=wk, in_=wk_d[:])
                        wv = attd.tile([128, VL * HD], BF16)
                        nc.sync.dma_start(out=wv, in_=wv_d[:])
                        bq = attd.tile([128, VL * 2], F32)
                        nc.sync.dma_start(out=bq, in_=bq_d[:])
                        bk = attd.tile([128, VL * 2], F32)
                        nc.sync.dma_start(out=bk, in_=bk_d[:])
                        bvo = attd.tile([1, VL * HD], BF16)
                        nc.sync.dma_start(out=bvo, in_=bvo_d[:])

                        qT = [attd.tile([128, 2 * N], BF16, name=f"qT{i}")
                              for i in range(VL)]
                        kTt = [attd.tile([128, 2 * N], BF16, name=f"kT{i}")
                               for i in range(VL)]
                        vB = [attd.tile([128, 2 * N], BF16, name=f"vB{i}")
                              for i in range(VL)]
                        with tc.tile_pool(name="prj", bufs=1,
                                          space="PSUM") as prj:
                            for i in range(VL):
                                for (wmat, bias, dstl, veng) in (
                                        (wq, bq, qT, False), (wk, bk, kTt, True)):
                                    pp = prj.tile([128, 2 * N], F32, tag="pp")
                                    for hg in range(2):
                                        for c in range(2):
                                            nc.tensor.matmul(
                                                pp[:, hg * N + c * 512:
                                                   hg * N + (c + 1) * 512],
                                                lhsT=wmat[
                                                    :, i * HD + hg * 128:
                                                    i * HD + (hg + 1) * 128],
                                                rhs=zA[i][:, c * 512:
                                                          (c + 1) * 512],
                                                start=True, stop=True,
                                            )
                                    for hg in range(2):
                                        if veng:
                                            nc.vector.tensor_scalar(
                                                out=dstl[i][:, hg * N:(hg + 1) * N],
                                                in0=pp[:, hg * N:(hg + 1) * N],
                                                scalar1=bias[:, i * 2 + hg:
                                                             i * 2 + hg + 1],
                                                scalar2=None, op0=ALU.add)
                                        else:
                                            nc.scalar.activation(
                                                dstl[i][:, hg * N:(hg + 1) * N],
                                                pp[:, hg * N:(hg + 1) * N],
                                                AF.Identity,
                                                bias=bias[:, i * 2 + hg:
                                                          i * 2 + hg + 1],
                                                scale=1.0)
                                # v in (token, padded-head) layout + bv + ones
                                vp = prj.tile([128, 2 * N], F32, tag="vp")
                                for nt in range(8):
                                    nc.tensor.matmul(
                                        vp[:, nt * 256:(nt + 1) * 256],
                                        lhsT=zA[i][:, nt * 128:(nt + 1) * 128],
                                        rhs=wv[:, i * HD:(i + 1) * HD],
                                        start=True, stop=False,
                                    )
                                    nc.tensor.matmul(
                                        vp[:, nt * 256:(nt + 1) * 256],
                                        lhsT=ones1,
                                        rhs=bvo[:, i * HD:(i + 1) * HD],
                                        start=False, stop=True,
                                    )
                                nc.vector.tensor_copy(vB[i], vp)

                        if _DEBUG:
                            with tc.tile_pool(name="dbgp", bufs=1) as dbgp:
                                dv = dbgp.tile([128, N], F32)
                                nc.vector.tensor_copy(dv, qT[0][:, 0:N])
                                nc.sync.dma_start(out=dbg_d[2], in_=dv)

                        # ---- attention core ----
                        with tc.tile_pool(name="att", bufs=6) as att, \
                             tc.tile_pool(name="scp", bufs=2,
                                          space="PSUM") as scp:
                            for b in range(B):
                                for i in range(VL):
                                    exps = []
                                    for kt in range(2):
                                        scP = scp.tile([128, 2048], F32,
                                                       tag="scP")
                                        for hh in range(4):
                                            for hg in range(2):
                                                jb = 2 * hh + hg
                                                nc.tensor.matmul(
                                                    scP[:, jb * 256:
                                                        (jb + 1) * 256],
                                                    lhsT=kTt[i][
                                                        32 * hh:32 * hh + 32,
                                                        hg * N + b * 256
                                                        + kt * 128:
                                                        hg * N + b * 256
                                                        + (kt + 1) * 128],
                                                    rhs=qT[i][
                                                        32 * hh:32 * hh + 32,
                                                        hg * N + b * 256:
                                                        hg * N + (b + 1) * 256],
                                                    start=True, stop=True,
                                                    tile_position=(32 * hh, 0),
                                                )
                                        ex = att.tile([128, 2048], BF16,
                                                      tag="ex")
                                        nc.scalar.activation(ex, scP, AF.Exp)
                                        exps.append(ex)
                                    otP = scp.tile([128, 512], F32,
                                                   tag="scP")
                                    for hg in range(2):
                                        for hh in range(4):
                                            j = 4 * hg + hh
                                            jb = 2 * hh + hg
                                            for kt in range(2):
                                                v0 = (b * 2 + kt) * 256 + j * 32
                                                nc.tensor.matmul(
                                                    otP[32 * hh:32 * hh + 32,
                                                        hg * 256:
                                                        (hg + 1) * 256],
                                                    lhsT=vB[i][:, v0:v0 + 32],
                                                    rhs=exps[kt][
                                                        :, jb * 256:
                                                        (jb + 1) * 256],
                                                    start=(kt == 0),
                                                    stop=(kt == 1),
                                                    tile_position=(0, 32 * hh),
                                                )
                                    nc.vector.tensor_copy(oT[b * VL + i], otP)
                                    if _DEBUG and b == 0 and i == 0:
                                        with tc.tile_pool(name="dbo",
                                                          bufs=1) as dbo:
                                            dvo = dbo.tile([128, 512], F32)
                                            nc.vector.tensor_copy(dvo, otP)
                                            nc.sync.dma_start(
                                                out=dbg_d[3].rearrange(
                                                    "p (a n) -> p a n", a=2)[
                                                    :, 0, :],
                                                in_=dvo)

                # ================= output head =================
                with tc.tile_pool(name="ph", bufs=2) as php, \
                     tc.tile_pool(name="phpp", bufs=1, space="PSUM") as phpp:
                    outst = php.tile([128, 64], F32)
                    for b in range(B):
                        PHp = phpp.tile([8, 2048], F32, tag="PHp")
                        PHl = phpp.tile([8, 2048], F32, tag="PHl")
                        for i in range(VL):
                            for hg in range(2):
                                nc.tensor.matmul(
                                    PHp[:, i * 256:(i + 1) * 256],
                                    lhsT=wfp[:, (i * 2 + hg) * 8:
                                             (i * 2 + hg + 1) * 8],
                                    rhs=oT[b * VL + i][:, hg * 256:
                                                       (hg + 1) * 256],
                                    start=(hg == 0), stop=(hg == 1),
                                )
                                nc.tensor.matmul(
                                    PHl[:, i * 256:(i + 1) * 256],
                                    lhsT=wfl[:, (i * 2 + hg) * 8:
                                             (i * 2 + hg + 1) * 8],
                                    rhs=oT[b * VL + i][:, hg * 256:
                                                       (hg + 1) * 256],
                                    start=(hg == 0), stop=(hg == 1),
                                )
                        PHs = php.tile([8, 2048], F32, tag="PHs")
                        nc.scalar.activation(PHs, PHp, AF.Copy)
                        rl = php.tile([8, 2048], F32, tag="rl")
                        nc.vector.reciprocal_approx_fast(out=rl, in_=PHl)
                        pn = php.tile([8, 2048], BF16, tag="pn")
                        nc.vector.tensor_tensor(
                            out=pn, in0=PHs, in1=rl, op=ALU.mult)
                        pP = phpp.tile([8, 256], F32, tag="PHp")
                        for i in range(VL):
                            nc.tensor.matmul(
                                pP[:, :],
                                lhsT=selHS[:, i * 8:(i + 1) * 8],
                                rhs=pn[:, i * 256:(i + 1) * 256],
                                start=(i == 0), stop=(i == VL - 1),
                            )
                        pSb = php.tile([8, 256], F32, tag="pSb")
                        nc.vector.tensor_scalar(
                            out=pSb, in0=pP, scalar1=bfold[:, 0:1],
                            scalar2=None, op0=ALU.add)
                        for c in range(2):
                            tp = phpp.tile([128, 8], F32, tag="PHl")
                            nc.tensor.transpose(
                                tp, pSb[:, c * 128:(c + 1) * 128], eye8)
                            nc.vector.tensor_copy(
                                outst[:, (b * 2 + c) * 8:(b * 2 + c + 1) * 8],
                                tp)
                    nc.sync.dma_start(
                        out=out_d.rearrange("(cb p) v -> p cb v", p=128),
                        in_=outst.rearrange("p (cb v) -> p cb v", v=VL))

    nc.compile()
    return nc


# ----------------------------------------------------------------------------
# Host-side preprocessing
# ----------------------------------------------------------------------------
def _prep_core(core, adj, var_emb, temp_emb, mech_W, mech_b, ln_g, ln_b,
               Wq, Wk, Wv, Wo, bq, bk, bv, bo, out_W, out_b, xT):
    import ml_dtypes
    bf = ml_dtypes.bfloat16
    gi = slice(core * VL, (core + 1) * VL)

    # U[(k), dl*64+s, i*H+h] = adj[s, g, 2k+dl] * (var_emb[s,h]+temp_emb[l,h])
    # folded with mech layer 0: Up = U @ W0, plus bias in the ones-row.
    Up = np.zeros((KT, 128, VL * H), np.float32)
    for i in range(VL):
        g = core * VL + i
        W0 = mech_W[g, 0]                                   # (H, H)
        for k in range(KT):
            for dl in range(2):
                lag = 2 * k + dl
                if lag > L:
                    continue
                emb = var_emb + temp_emb[lag][None, :]      # (64, H)
                a = adj[:, g, lag]                          # (64,)
                blk = (a[:, None] * emb) @ W0               # (64, H)
                Up[k, dl * 64:dl * 64 + 64, i * H:(i + 1) * H] = blk
        Up[KT - 1, 64, i * H:(i + 1) * H] = mech_b[g, 0]

    mw2 = np.zeros((128, 2 * VL * H), np.float32)
    mb2 = np.zeros((128, 2 * VL), np.float32)
    lng = np.zeros((128, NL * VL), np.float32)
    lnb = np.zeros((128, NL * VL), np.float32)
    for li in range(NL):
        for i in range(VL):
            g = core * VL + i
            lng[:, li * VL + i] = ln_g[g, li]
            lnb[:, li * VL + i] = ln_b[g, li]
            if li >= 1:
                ci = (li - 1) * VL + i
                mw2[:, ci * H:(ci + 1) * H] = mech_W[g, li]
                mb2[:, (li - 1) * VL + i] = mech_b[g, li]

    selMu = np.zeros((128, 64), np.float32)
    selSS = np.zeros((128, 64), np.float32)
    for i in range(8):
        selMu[:, i * 8 + i] = 1.0 / H
        selSS[:, i * 8 + i] = 1.0 / H

    def padw(Wm, scale=1.0):
        # (VL,H,H) -> (128, VL*HD): col i*HD + 32*j + d = W[:, 16j+d]*scale
        out = np.zeros((128, VL * HD), np.float32)
        for i in range(VL):
            g = core * VL + i
            for j in range(NH):
                out[:, i * HD + 32 * j:i * HD + 32 * j + DH] = \
                    Wm[g][:, DH * j:DH * (j + 1)] * scale
        return out

    wqp = padw(Wq, 1.0 / np.sqrt(DH))
    wkp = padw(Wk)
    wvp = padw(Wv)

    def padb(bm, scale=1.0):
        # (VL,H) -> (128, VL*2) per-partition cols by (i, hg)
        out = np.zeros((128, VL * 2), np.float32)
        for i in range(VL):
            g = core * VL + i
            for j in range(NH):
                hg, hh = divmod(j, 4)
                out[32 * hh:32 * hh + DH, i * 2 + hg] = \
                    bm[g][DH * j:DH * (j + 1)] * scale
        return out

    bqp = padb(bq, 1.0 / np.sqrt(DH))
    bkp = padb(bk)
    bvo = np.zeros((1, VL * HD), np.float32)
    for i in range(VL):
        g = core * VL + i
        for j in range(NH):
            bvo[0, i * HD + 32 * j:i * HD + 32 * j + DH] = \
                bv[g][DH * j:DH * (j + 1)]
            bvo[0, i * HD + 32 * j + DH] = 1.0   # ones column -> softmax denom

    wfold = np.einsum('vhk,vk->vh', Wo[gi], out_W[gi])      # (VL, H)
    wfp = np.zeros((128, VL * 2 * 8), np.float32)
    wfl = np.zeros((128, VL * 2 * 8), np.float32)
    for i in range(VL):
        for hg in range(2):
            m0 = (i * 2 + hg) * 8
            for hh in range(4):
                j = 4 * hg + hh
                wfp[32 * hh:32 * hh + DH, m0 + j] = \
                    wfold[i, DH * j:DH * (j + 1)]
                wfl[32 * hh + DH, m0 + j] = 1.0             # l extraction
    bfold = (np.einsum('vh,vh->v', bo[gi], out_W[gi]) +
             out_b[gi]).astype(np.float32).reshape(8, 1)

    selHS = np.zeros((8, VL * 8), np.float32)
    for i in range(VL):
        selHS[:, i * 8 + i] = 1.0
    ones1 = np.ones((1, 128), np.float32)
    eye8 = np.eye(8, dtype=np.float32)

    bfc = lambda a: a.astype(bf)
    return {
        "xT": xT, "Up": bfc(Up), "mw2": bfc(mw2), "mb2": mb2,
        "lng": lng, "lnb": lnb,
        "selMu": bfc(selMu), "selSS": bfc(selSS),
        "wq": bfc(wqp), "wk": bfc(wkp), "wv": bfc(wvp), "bq": bqp, "bk": bkp,
        "bvo": bfc(bvo), "ones1": bfc(ones1), "wfp": bfc(wfp),
        "wfl": bfc(wfl), "selHS": bfc(selHS), "eye8": eye8, "bfold": bfold,
    }


def _run(inputs, trace=False):
    global _CACHED
    if _CACHED is None:
        _CACHED = _build_graph()
    nc = _CACHED

    f = lambda t: np.asarray(t, np.float32)
    x = f(inputs["x"])
    adj = 1.0 / (1.0 + np.exp(-f(inputs["adjacency_logits"])))
    xT = np.ascontiguousarray(
        x.reshape(N, V).T).astype(np.float32)            # (64, 1024)

    args = dict(
        adj=adj, var_emb=f(inputs["var_emb"]), temp_emb=f(inputs["temp_emb"]),
        mech_W=f(inputs["mech_W"]), mech_b=f(inputs["mech_b"]),
        ln_g=f(inputs["ln_g"]), ln_b=f(inputs["ln_b"]),
        Wq=f(inputs["Wq"]), Wk=f(inputs["Wk"]), Wv=f(inputs["Wv"]),
        Wo=f(inputs["Wo"]), bq=f(inputs["bq"]), bk=f(inputs["bk"]),
        bv=f(inputs["bv"]), bo=f(inputs["bo"]),
        out_W=f(inputs["out_W"]), out_b=f(inputs["out_b"]), xT=xT,
    )
    in_maps = [_prep_core(c, **args) for c in range(NCORES)]
    res = run_bass_kernel_spmd(nc, in_maps, list(range(NCORES)), trace=trace)
    preds = np.concatenate(
        [res.results[c]["preds"].reshape(B, S, VL) for c in range(NCORES)],
        axis=2).astype(np.float32)
    return preds, res


def kernel(**inputs):
    preds, _ = _run(inputs, trace=False)
    return preds
